# revision 1
# baseline (speedup 1.0000x reference)
"""LocalScoreMachine Trainium2 kernel.

score[b,c,p] = -sum_n w[b,n,p]*(x[b,c,p]-m*I[n,c,p]) / (sig2 * sum_n w[b,n,p])
with w = exp(-box3(|x - m*I|^2 summed over c)/(2*sig2) - sub).

Expansion: box3(norm) = box3(A) + m^2*box3(S) - 2m*box3(z),
A = sum_c x_c^2 (b-only), S = sum_c I_c^2 (n-only), z = sum_c x_c*I_c.
The exp factor from box3(A) (and any per-(b,p) stabilizer) is constant in n,c
and cancels in the numerator/denominator ratio, so each core computes
    w' = exp(box3((m/sig2)*z - (m^2/(2 sig2))*S))
over its shard of N, accumulates SW = sum_n w', SWI_c = sum_n w'*I_c via
TensorE ones-matmuls, and the host combines 8 partial results:
    score = (m*SWI/SW - x)/sig2.

Sharding: dataset axis N=2048 -> 256 images per core (8 cores), as 2 tiles of
[128 partitions = n, (3,32,32) free].
"""

import sys

for _p in ("/opt/trn_rl_repo", "/opt/trn_rl_repo/concourse", "/opt/pypackages"):
    if _p not in sys.path:
        sys.path.append(_p)

from contextlib import ExitStack

import numpy as np

import concourse.bass as bass
import concourse.bacc as bacc
import concourse.mybir as mybir
import concourse.tile as tile
from concourse import bass_utils

B, N, C, H, W = 8, 2048, 3, 32, 32
P = H * W  # 1024 pixels
NCORES = 8
NLOC = N // NCORES  # 256
NT = NLOC // 128  # 2 partition tiles per core
F32 = mybir.dt.float32
AF = mybir.ActivationFunctionType

_cache = {}
_last_res = None


def _build(m: float, sig2: float):
    """Build + compile the per-core SPMD program. m, sig2 are compile-time."""
    nc = bacc.Bacc("TRN2", target_bir_lowering=False, debug=False)

    img_d = nc.dram_tensor("img", [NLOC, C, H, W], F32, kind="ExternalInput")
    xs_d = nc.dram_tensor("xs", [B, C * P], F32, kind="ExternalInput")
    out_d = nc.dram_tensor("out", [B, 4, P], F32, kind="ExternalOutput")

    c_s = -(m * m) / (2.0 * sig2)  # multiplies S
    # z scale m/sig2 is folded into xs on the host.

    with tile.TileContext(nc) as tc, ExitStack() as ctx:
        const = ctx.enter_context(tc.tile_pool(name="const", bufs=1))
        imgs = ctx.enter_context(tc.tile_pool(name="imgs", bufs=1))
        spool = ctx.enter_context(tc.tile_pool(name="spool", bufs=1))
        pre = ctx.enter_context(tc.tile_pool(name="pre", bufs=1))
        xrs_pool = ctx.enter_context(tc.tile_pool(name="xrs", bufs=4))
        workv = ctx.enter_context(tc.tile_pool(name="workv", bufs=2))
        work = ctx.enter_context(tc.tile_pool(name="work", bufs=3))
        psum = ctx.enter_context(
            tc.tile_pool(name="psum", bufs=2, space=bass.MemorySpace.PSUM)
        )
        acc_pool = ctx.enter_context(
            tc.tile_pool(name="acc", bufs=1, space=bass.MemorySpace.PSUM)
        )

        ones_row = const.tile([1, 128], F32)  # lhsT for broadcast (K=1,M=128)
        ones_col = const.tile([128, 32], F32)  # lhsT for reduction (K=128,M=32)
        nc.gpsimd.memset(ones_row[:], 1.0)
        nc.gpsimd.memset(ones_col[:], 1.0)



        img_ap = img_d.ap().rearrange("(t p) c h w -> t p (c h w)", p=128)
        itiles = []
        spp = []
        for t in range(NT):
            it = imgs.tile([128, C, P], F32, tag=f"img{t}", name=f"img{t}")
            nc.sync.dma_start(it[:], img_ap[t])
            itiles.append(it)

            # S'' = c_s * sum_c I_c^2
            sq = pre.tile([128, C, P], F32, tag="sq")
            nc.scalar.square(sq[:], it[:])
            s0 = work.tile([128, P], F32, tag="tmp")
            nc.vector.tensor_add(s0[:], sq[:, 0], sq[:, 1])
            s1 = work.tile([128, P], F32, tag="chain")
            nc.vector.tensor_add(s1[:], s0[:], sq[:, 2])
            sp = spool.tile([128, P], F32, tag=f"spp{t}", name=f"spp{t}")
            nc.vector.tensor_scalar_mul(sp[:], s1[:], c_s)
            spp.append(sp)

        for b in range(B):
            # stage xs[b] on partition 0, then broadcast via PE ones-matmul
            xsb = workv.tile([1, C * P], F32, tag="xsb", name=f"xsb_{b}")
            nc.sync.dma_start(xsb[:], xs_d.ap()[b][None, :])
            xrc = []
            for c in range(C):
                xp = psum.tile([128, P], F32, tag="xr", name=f"xr_{b}_{c}")
                for half in range(2):
                    nc.tensor.matmul(
                        xp[:, half * 512 : (half + 1) * 512],
                        ones_row[:],
                        xsb[0:1, c * P + half * 512 : c * P + half * 512 + 512],
                    )
                xr_sb = xrs_pool.tile([128, P], F32, tag="xrs", name=f"xrs_{b}_{c}")
                nc.scalar.copy(xr_sb[:], xp[:])
                xrc.append(xr_sb)

            # accumulators: quadrant-packed redundant-row [32,512] blocks
            # accq[half] rows: 0-31=SW, 32-63=SWI0, 64-95=SWI1; accr[half]=SWI2
            accq0 = acc_pool.tile([96, 512], F32, tag="accq0")
            accq1 = acc_pool.tile([96, 512], F32, tag="accq1")
            accr0 = acc_pool.tile([32, 512], F32, tag="accr0")
            accr1 = acc_pool.tile([32, 512], F32, tag="accr1")
            accq = [accq0, accq1]
            accr = [accr0, accr1]

            for t in range(NT):
                it = itiles[t]
                # u = S'' + sum_c I_c * xs_c   (xs pre-scaled by m/sig2)
                t0 = work.tile([128, P], F32, tag="tmp")
                nc.vector.tensor_mul(t0[:], it[:, 0], xrc[0][:])
                u0 = work.tile([128, P], F32, tag="chain")
                nc.vector.tensor_add(u0[:], t0[:], spp[t][:])
                t1 = work.tile([128, P], F32, tag="tmp")
                nc.vector.tensor_mul(t1[:], it[:, 1], xrc[1][:])
                u1 = work.tile([128, P], F32, tag="chain")
                nc.vector.tensor_add(u1[:], u0[:], t1[:])
                t2 = work.tile([128, P], F32, tag="tmp")
                nc.vector.tensor_mul(t2[:], it[:, 2], xrc[2][:])
                u = work.tile([128, H, W], F32, tag="chain")
                nc.vector.tensor_add(
                    u[:].rearrange("p h w -> p (h w)"), u1[:], t2[:]
                )

                # separable 3x3 box filter (zero pad), free dims (h, w)
                r = work.tile([128, H, W], F32, tag="tmp")  # t[w] = u[w]+u[w+1]
                nc.vector.tensor_add(r[:, :, 0:31], u[:, :, 0:31], u[:, :, 1:32])
                nc.scalar.copy(r[:, :, 31:32], u[:, :, 31:32])
                r2 = work.tile([128, H, W], F32, tag="chain")  # rowsum
                nc.vector.tensor_add(r2[:, :, 1:32], r[:, :, 1:32], u[:, :, 0:31])
                nc.scalar.copy(r2[:, :, 0:1], r[:, :, 0:1])

                s = work.tile([128, H, W], F32, tag="tmp")  # t2[h] = r2[h]+r2[h+1]
                nc.vector.tensor_add(s[:, 0:31, :], r2[:, 0:31, :], r2[:, 1:32, :])
                nc.scalar.copy(s[:, 31:32, :], r2[:, 31:32, :])
                arg = work.tile([128, H, W], F32, tag="chain")  # full box sum
                nc.vector.tensor_add(arg[:, 1:32, :], s[:, 1:32, :], r2[:, 0:31, :])
                nc.scalar.copy(arg[:, 0:1, :], s[:, 0:1, :])

                wt = work.tile([128, H, W], F32, tag="wt")
                nc.scalar.activation(wt[:], arg[:], AF.Exp)

                v = workv.tile([128, C, P], F32, tag="v")
                wflat = wt[:].rearrange("p h w -> p (h w)")
                for c in range(C):
                    nc.vector.tensor_mul(v[:, c], wflat, it[:, c])

                # reduce over n (partitions) via ones matmuls, accumulate in PSUM
                first, last = (t == 0), (t == NT - 1)
                for half in range(2):
                    sl = slice(half * 512, (half + 1) * 512)
                    nc.tensor.matmul(
                        accq[half][0:32], ones_col[:], wflat[:, sl],
                        start=first, stop=last,
                    )
                    nc.tensor.matmul(
                        accq[half][32:64], ones_col[:], v[:, 0, sl],
                        start=first, stop=last,
                    )
                    nc.tensor.matmul(
                        accq[half][64:96], ones_col[:], v[:, 1, sl],
                        start=first, stop=last,
                    )
                    nc.tensor.matmul(
                        accr[half][0:32], ones_col[:], v[:, 2, sl],
                        start=first, stop=last,
                    )

            for half in range(2):
                sl = slice(half * 512, (half + 1) * 512)
                oq = work.tile([96, 512], F32, tag="oq", name=f"oq_{b}_{half}")
                nc.scalar.copy(oq[:], accq[half][:])
                orr = work.tile([32, 512], F32, tag="orr", name=f"orr_{b}_{half}")
                nc.scalar.copy(orr[:], accr[half][:])
                nc.sync.dma_start(out_d.ap()[b, 0, sl], oq[0:1, :])
                nc.sync.dma_start(out_d.ap()[b, 1, sl], oq[32:33, :])
                nc.sync.dma_start(out_d.ap()[b, 2, sl], oq[64:65, :])
                nc.sync.dma_start(out_d.ap()[b, 3, sl], orr[0:1, :])

    nc.compile()
    return nc


def kernel(x, images, mu, sigma, t):
    x = np.ascontiguousarray(np.asarray(x, dtype=np.float32))
    images = np.ascontiguousarray(np.asarray(images, dtype=np.float32))
    m = float(np.asarray(mu)[int(t)])
    sig = float(np.asarray(sigma)[int(t)])
    sig2 = sig * sig

    key = (m, sig2)
    if key not in _cache:
        _cache[key] = _build(m, sig2)
    nc = _cache[key]

    xs = (x.reshape(B, C * P) * (m / sig2)).astype(np.float32)
    imgs = images.reshape(N, C * P)
    in_maps = []
    for k in range(NCORES):
        in_maps.append(
            {
                "img": np.ascontiguousarray(
                    imgs[k * NLOC : (k + 1) * NLOC].reshape(NLOC, C, H, W)
                ),
                "xs": xs,
            }
        )

    import os
    trace = bool(os.environ.get("KERNEL_TRACE"))
    res = bass_utils.run_bass_kernel_spmd(
        nc, in_maps, core_ids=list(range(NCORES)), trace=trace
    )
    global _last_res
    _last_res = res
    parts = np.stack([res.results[k]["out"] for k in range(NCORES)])  # [8,B,4,P]
    tot = parts.sum(axis=0)
    sw = tot[:, 0, :]  # [B,P]
    swi = tot[:, 1:4, :]  # [B,C,P]
    score = (m * swi / sw[:, None, :] - x.reshape(B, C, P)) / sig2
    return score.reshape(B, C, H, W).astype(np.float32)



# revision 3
# speedup vs baseline: 1.1145x; 1.1145x over previous
"""LocalScoreMachine Trainium2 kernel, v3: K-stacked box matmuls.

arg[b, po, n] = sum_{c,pi} B[pi,po] * xhat_bc[pi] * I_c[pi,n]  (+ S'' as c=3,
xhat=1). Rhs shipped as row-ktiles: one SBUF tile per image row r holding
[(4 channels x 32 pixels) = 128 partitions, n]. Each output tile t contracts
its 6 halo rows with 6 per-(b,t) lhsT matrices built by a single 4x
tensor_scalar row-scaling of a constant band pattern. This removes all
z-products and the S'' add from DVE and cuts PE to 6 matmuls per chunk.
w = exp(arg) on ACT (free SW accum); SWI_c via DVE products + ts accums.
Sharding: 4 query-pair groups x 2 dataset halves.
"""

import sys

for _p in ("/opt/trn_rl_repo", "/opt/trn_rl_repo/concourse", "/opt/pypackages"):
    if _p not in sys.path:
        sys.path.append(_p)

from contextlib import ExitStack

import numpy as np

import concourse.bass as bass
import concourse.bacc as bacc
import concourse.mybir as mybir
import concourse.tile as tile
from concourse import bass_utils

B, N, C, H, W = 8, 2048, 3, 32, 32
P = H * W
NCORES = 8
GB, GN = 4, 2
NB = B // GB  # 2
NLOC = N // GN  # 1024
PT = P // 128  # 8
F = NLOC
CH = 512

F32 = mybir.dt.float32
F16 = mybir.dt.float16
AF = mybir.ActivationFunctionType
OP = mybir.AluOpType

ACT_MOD = 2  # every ACT_MOD-th channel reduction goes to DVE, rest to ACT

_cache = {}
_last_res = None


def _build():
    nc = bacc.Bacc("TRN2", target_bir_lowering=False, debug=False)

    rows_d = nc.dram_tensor("rows", [H, 128, F], F16, kind="ExternalInput")
    img_d = nc.dram_tensor("imgs", [PT, 128, 3, F], F16, kind="ExternalInput")
    xs_d = nc.dram_tensor("xs", [128, H * NB], F32, kind="ExternalInput")
    bs_d = nc.dram_tensor("bstk", [6, 128, 128], F16, kind="ExternalInput")
    out_d = nc.dram_tensor("out", [128, NB * PT * 4], F32, kind="ExternalOutput")

    with tile.TileContext(nc) as tc, ExitStack() as ctx:
        const = ctx.enter_context(tc.tile_pool(name="const", bufs=1))
        rpool = ctx.enter_context(tc.tile_pool(name="rpool", bufs=1))
        imgs = ctx.enter_context(tc.tile_pool(name="imgs", bufs=1))
        lp = ctx.enter_context(tc.tile_pool(name="lp", bufs=2))
        wp = ctx.enter_context(tc.tile_pool(name="wp", bufs=3))
        scr = ctx.enter_context(tc.tile_pool(name="scr", bufs=3))
        psum = ctx.enter_context(
            tc.tile_pool(name="psum", bufs=2, space=bass.MemorySpace.PSUM)
        )

        bstk = const.tile([128, 6 * 128], F16)
        nc.scalar.dma_start(
            bstk[:].rearrange("p (k m) -> p k m", k=6),
            bs_d.ap().rearrange("k p m -> p k m"),
        )
        xst = const.tile([128, H * NB], F32)
        nc.scalar.dma_start(xst[:], xs_d.ap())
        stage = const.tile([128, NB * PT * 4], F32)

        # row-ktiles batched 4 rows per DMA (Pool queue); image tiles on ACT's
        rtiles = []
        itiles = []
        for g in range(PT):
            gt = rpool.tile([128, 4 * F], F16, tag=f"rg{g}", name=f"rg{g}")
            nc.gpsimd.dma_start(
                gt[:].rearrange("p (r n) -> p r n", r=4),
                rows_d.ap()[4 * g : 4 * g + 4].rearrange("r p n -> p r n"),
            )
            for j in range(4):
                rtiles.append(gt[:, j * F : (j + 1) * F])
            it = imgs.tile([128, 3 * F], F16, tag=f"img{g}", name=f"img{g}")
            nc.scalar.dma_start(
                it[:].rearrange("p (c n) -> p c n", c=3), img_d.ap()[g]
            )
            itiles.append(it)

        def xcol(b, r):
            j = r * NB + b
            return xst[:, j : j + 1]

        def rows_of(t):
            return [r for r in range(4 * t - 1, 4 * t + 5) if 0 <= r < H]

        lts = [[None] * PT for _ in range(NB)]

        def emit_lhs(b, t):
            ls = []
            for r in rows_of(t):
                j = r - (4 * t - 1)  # band-pattern index 0..5
                L = lp.tile([128, 128], F16, tag=f"L{b}_{j}", name=f"L{b}_{j}_{t}")
                nc.vector.tensor_scalar_mul(
                    L[:], bstk[:, j * 128 : (j + 1) * 128], xcol(b, r)
                )
                ls.append((r, L))
            lts[b][t] = ls

        def scol(b, t, j):
            k = b * (PT * 4) + t * 4 + j
            return stage[:, k : k + 1]

        wts = [[None] * PT for _ in range(NB)]

        def emit_boxexp(b, t):
            ls = lts[b][t]
            ps = psum.tile([128, F], F32, tag=f"ps{b}", name=f"ps_{b}_{t}")
            n_mm = len(ls) * 2
            i = 0
            for r, L in ls:
                for ck in range(2):
                    sl = slice(ck * CH, (ck + 1) * CH)
                    nc.tensor.matmul(
                        ps[:, sl],
                        L[:],
                        rtiles[r][:, sl.start : sl.stop] if False else rtiles[r][:, sl],
                        start=(i < 2),
                        stop=(i >= n_mm - 2),
                        skip_group_check=True,
                    )
                    i += 1
            w = wp.tile([128, F], F16, tag=f"w{b}", name=f"w_{b}_{t}")
            nc.scalar.activation(w[:], ps[:], AF.Exp, accum_out=scol(b, t, 0))
            wts[b][t] = w

        rctr = [0]

        def emit_red(b, t, kact=None):
            it = itiles[t]
            w = wts[b][t]
            for c in range(C):
                v = scr.tile([128, F], F16, tag=f"v_{b}", name=f"v{c}_{b}_{t}")
                nc.vector.tensor_tensor(
                    v[:], w[:], it[:, c * F : (c + 1) * F], OP.mult
                )
                rctr[0] += 1
                use_act = (rctr[0] % ACT_MOD) != 0 if kact is None else False
                if use_act:
                    d = scr.tile([128, F], F16, tag="da", name=f"d{c}_{b}_{t}")
                    nc.scalar.activation(
                        d[:], v[:], AF.Copy, accum_out=scol(b, t, 1 + c)
                    )
                else:
                    d = scr.tile([128, F], F16, tag="dv", name=f"d{c}_{b}_{t}")
                    nc.vector.tensor_scalar(
                        d[:], v[:], 1.0, 0.0, OP.mult, OP.add,
                        accum_out=scol(b, t, 1 + c),
                    )

        steps = [(t, b) for t in range(PT) for b in range(NB)]
        nst = len(steps)
        for k in range(nst + 4):
            if k < nst:
                t, b = steps[k]
                emit_lhs(b, t)
            if 2 <= k < nst + 2:
                t1_, b1_ = steps[k - 2]
                emit_boxexp(b1_, t1_)
            if 4 <= k < nst + 4:
                t2_, b2_ = steps[k - 4]
                emit_red(b2_, t2_, kact=0 if k >= nst + 2 else None)

        nc.sync.dma_start(out_d.ap(), stage[:])

    nc.compile()
    return nc


def _band_stack():
    # bstk[j][(c,px_in), po] : band for input row-offset j-1 vs output row po//32
    px_i = np.arange(128) % 32  # within each 32-px channel group
    po = np.arange(128)
    ro, pxo = po // 32, po % 32
    b6 = np.zeros((6, 128, 128), np.float16)
    for j in range(6):
        rd = (j - 1) - ro[None, :]
        wd = px_i[:, None] - pxo[None, :]
        b6[j] = ((np.abs(rd) <= 1) & (np.abs(wd) <= 1)).astype(np.float16)
    return b6


def kernel(x, images, mu, sigma, t):
    x = np.ascontiguousarray(np.asarray(x, dtype=np.float32))
    images = np.ascontiguousarray(np.asarray(images, dtype=np.float32))
    m = float(np.asarray(mu)[int(t)])
    sig = float(np.asarray(sigma)[int(t)])
    sig2 = sig * sig

    if "nc" not in _cache:
        _cache["nc"] = _build()
    nc = _cache["nc"]

    imgs2 = images.reshape(N, C, P)
    spp = (-(m * m) / (2.0 * sig2)) * np.einsum("ncp,ncp->np", imgs2, imgs2)
    xx = (x.reshape(B, C, H, W) * (m / sig2)).astype(np.float32)
    b6 = _band_stack()

    in_maps = []
    for k in range(NCORES):
        bg, nh = k // GN, k % GN
        nsl = slice(nh * NLOC, (nh + 1) * NLOC)
        # rows[r]: [(c,px), n] ; c<3 = I_c row r, c=3 = S'' row r
        rows = np.empty((H, 4, 32, F), np.float16)
        for c in range(C):
            rows[:, c] = (
                imgs2[nsl, c, :].T.reshape(H, 32, F)
            )
        rows[:, 3] = spp[nsl].T.reshape(H, 32, F)
        arr = np.empty((PT, 128, 3, F), np.float16)
        for c in range(C):
            arr[:, :, c, :] = imgs2[nsl, c, :].T.reshape(PT, 128, F)
        # xs[(c,px), r*NB+b] = xhat_{b,c}[row r, px]; 1.0 for c=3
        xs = np.ones((4, 32, H, NB), np.float32)
        for c in range(C):
            for b in range(NB):
                xs[c, :, :, b] = xx[bg * NB + b, c].T  # [W=32 px, H=32 rows]
        xs = xs.reshape(128, H * NB)
        in_maps.append(
            {
                "rows": np.ascontiguousarray(rows.reshape(H, 128, F)),
                "imgs": np.ascontiguousarray(arr),
                "xs": np.ascontiguousarray(xs),
                "bstk": b6,
            }
        )

    import os

    trace = bool(os.environ.get("KERNEL_TRACE"))
    res = bass_utils.run_bass_kernel_spmd(
        nc, in_maps, core_ids=list(range(NCORES)), trace=trace
    )
    global _last_res
    _last_res = res

    sw = np.zeros((B, P), np.float64)
    swi = np.zeros((B, C, P), np.float64)
    for k in range(NCORES):
        bg = k // GN
        st = np.asarray(res.results[k]["out"], np.float64).reshape(128, NB, PT, 4)
        for b in range(NB):
            gb = bg * NB + b
            sw[gb] += st[:, b, :, 0].T.reshape(P)
            for c in range(C):
                swi[gb, c] += st[:, b, :, 1 + c].T.reshape(P)

    score = (m * swi / sw[:, None, :] - x.reshape(B, C, P)) / sig2
    return score.reshape(B, C, H, W).astype(np.float32)


# revision 4
# speedup vs baseline: 1.1350x; 1.0184x over previous
"""LocalScoreMachine Trainium2 kernel, v3: K-stacked box matmuls.

arg[b, po, n] = sum_{c,pi} B[pi,po] * xhat_bc[pi] * I_c[pi,n]  (+ S'' as c=3,
xhat=1). Rhs shipped as row-ktiles: one SBUF tile per image row r holding
[(4 channels x 32 pixels) = 128 partitions, n]. Each output tile t contracts
its 6 halo rows with 6 per-(b,t) lhsT matrices built by a single 4x
tensor_scalar row-scaling of a constant band pattern. This removes all
z-products and the S'' add from DVE and cuts PE to 6 matmuls per chunk.
w = exp(arg) on ACT (free SW accum); SWI_c via DVE products + ts accums.
Sharding: 4 query-pair groups x 2 dataset halves.
"""

import sys

for _p in ("/opt/trn_rl_repo", "/opt/trn_rl_repo/concourse", "/opt/pypackages"):
    if _p not in sys.path:
        sys.path.append(_p)

from contextlib import ExitStack

import numpy as np

import concourse.bass as bass
import concourse.bacc as bacc
import concourse.mybir as mybir
import concourse.tile as tile
from concourse import bass_utils

B, N, C, H, W = 8, 2048, 3, 32, 32
P = H * W
NCORES = 8
GB, GN = 4, 2
NB = B // GB  # 2
NLOC = N // GN  # 1024
PT = P // 128  # 8
F = NLOC
CH = 512

F32 = mybir.dt.float32
F16 = mybir.dt.float16
AF = mybir.ActivationFunctionType
OP = mybir.AluOpType

ACT_MOD = 2  # every ACT_MOD-th channel reduction goes to DVE, rest to ACT

_cache = {}
_last_res = None


def _build():
    nc = bacc.Bacc("TRN2", target_bir_lowering=False, debug=False)

    rows_d = nc.dram_tensor("rows", [H, 128, F], F16, kind="ExternalInput")
    img_d = nc.dram_tensor("imgs", [PT, 128, 3, F], F16, kind="ExternalInput")
    xs_d = nc.dram_tensor("xs", [128, H * NB], F32, kind="ExternalInput")
    bs_d = nc.dram_tensor("bstk", [6, 128, 128], F16, kind="ExternalInput")
    out_d = nc.dram_tensor("out", [128, NB * PT * 4], F32, kind="ExternalOutput")

    with tile.TileContext(nc) as tc, ExitStack() as ctx:
        const = ctx.enter_context(tc.tile_pool(name="const", bufs=1))
        rpool = ctx.enter_context(tc.tile_pool(name="rpool", bufs=1))
        imgs = ctx.enter_context(tc.tile_pool(name="imgs", bufs=1))
        lp = ctx.enter_context(tc.tile_pool(name="lp", bufs=2))
        wp = ctx.enter_context(tc.tile_pool(name="wp", bufs=3))
        scr = ctx.enter_context(tc.tile_pool(name="scr", bufs=3))
        psum = ctx.enter_context(
            tc.tile_pool(name="psum", bufs=2, space=bass.MemorySpace.PSUM)
        )

        bstk = const.tile([128, 6 * 128], F16)
        nc.scalar.dma_start(
            bstk[:].rearrange("p (k m) -> p k m", k=6),
            bs_d.ap().rearrange("k p m -> p k m"),
        )
        xst = const.tile([128, H * NB], F32)
        nc.scalar.dma_start(xst[:], xs_d.ap())
        stage = const.tile([128, NB * PT * 4], F32)

        # row-ktiles batched 4 rows per DMA (Pool queue); first group split in
        # halves so the tiny bstk/xs transfers are not stuck behind it
        rtiles = []
        itiles = []
        for g in range(PT):
            gt = rpool.tile([128, 4 * F], F16, tag=f"rg{g}", name=f"rg{g}")
            if g == 0:
                for h in range(2):
                    nc.gpsimd.dma_start(
                        gt[:, h * 2 * F : (h + 1) * 2 * F].rearrange(
                            "p (r n) -> p r n", r=2
                        ),
                        rows_d.ap()[2 * h : 2 * h + 2].rearrange("r p n -> p r n"),
                    )
            else:
                nc.gpsimd.dma_start(
                    gt[:].rearrange("p (r n) -> p r n", r=4),
                    rows_d.ap()[4 * g : 4 * g + 4].rearrange("r p n -> p r n"),
                )
            for j in range(4):
                rtiles.append(gt[:, j * F : (j + 1) * F])
            it = imgs.tile([128, 3 * F], F16, tag=f"img{g}", name=f"img{g}")
            itiles.append(it)

        def emit_img_dma(g):
            nc.scalar.dma_start(
                itiles[g][:].rearrange("p (c n) -> p c n", c=3), img_d.ap()[g]
            )

        def xcol(b, r):
            j = r * NB + b
            return xst[:, j : j + 1]

        def rows_of(t):
            return [r for r in range(4 * t - 1, 4 * t + 5) if 0 <= r < H]

        lts = [[None] * PT for _ in range(NB)]

        def emit_lhs(b, t):
            ls = []
            for r in rows_of(t):
                j = r - (4 * t - 1)  # band-pattern index 0..5
                L = lp.tile([128, 128], F16, tag=f"L{b}_{j}", name=f"L{b}_{j}_{t}")
                nc.vector.tensor_scalar_mul(
                    L[:], bstk[:, j * 128 : (j + 1) * 128], xcol(b, r)
                )
                ls.append((r, L))
            lts[b][t] = ls

        def scol(b, t, j):
            k = b * (PT * 4) + t * 4 + j
            return stage[:, k : k + 1]

        wts = [[None] * PT for _ in range(NB)]

        def emit_boxexp(b, t):
            ls = lts[b][t]
            ps = psum.tile([128, F], F32, tag=f"ps{b}", name=f"ps_{b}_{t}")
            n_mm = len(ls) * 2
            i = 0
            for r, L in ls:
                for ck in range(2):
                    sl = slice(ck * CH, (ck + 1) * CH)
                    nc.tensor.matmul(
                        ps[:, sl],
                        L[:],
                        rtiles[r][:, sl.start : sl.stop] if False else rtiles[r][:, sl],
                        start=(i < 2),
                        stop=(i >= n_mm - 2),
                        skip_group_check=True,
                    )
                    i += 1
            w = wp.tile([128, F], F16, tag=f"w{b}", name=f"w_{b}_{t}")
            nc.scalar.activation(w[:], ps[:], AF.Exp, accum_out=scol(b, t, 0))
            wts[b][t] = w

        rctr = [0]

        def emit_red(b, t, kact=None):
            it = itiles[t]
            w = wts[b][t]
            for c in range(C):
                v = scr.tile([128, F], F16, tag=f"v_{b}", name=f"v{c}_{b}_{t}")
                nc.vector.tensor_tensor(
                    v[:], w[:], it[:, c * F : (c + 1) * F], OP.mult
                )
                rctr[0] += 1
                use_act = (rctr[0] % ACT_MOD) != 0 if kact is None else False
                if use_act:
                    d = scr.tile([128, F], F16, tag="da", name=f"d{c}_{b}_{t}")
                    nc.scalar.activation(
                        d[:], v[:], AF.Copy, accum_out=scol(b, t, 1 + c)
                    )
                else:
                    d = scr.tile([128, F], F16, tag="dv", name=f"d{c}_{b}_{t}")
                    nc.vector.tensor_scalar(
                        d[:], v[:], 1.0, 0.0, OP.mult, OP.add,
                        accum_out=scol(b, t, 1 + c),
                    )

        steps = [(t, b) for t in range(PT) for b in range(NB)]
        nst = len(steps)
        for k in range(nst + 4):
            if k < nst:
                t, b = steps[k]
                if b == 0:
                    emit_img_dma(t)
                emit_lhs(b, t)
            if 2 <= k < nst + 2:
                t1_, b1_ = steps[k - 2]
                emit_boxexp(b1_, t1_)
            if 4 <= k < nst + 4:
                t2_, b2_ = steps[k - 4]
                emit_red(b2_, t2_, kact=0 if k >= nst + 2 else None)

        nc.sync.dma_start(out_d.ap(), stage[:])

    nc.compile()
    return nc


def _band_stack():
    # bstk[j][(c,px_in), po] : band for input row-offset j-1 vs output row po//32
    px_i = np.arange(128) % 32  # within each 32-px channel group
    po = np.arange(128)
    ro, pxo = po // 32, po % 32
    b6 = np.zeros((6, 128, 128), np.float16)
    for j in range(6):
        rd = (j - 1) - ro[None, :]
        wd = px_i[:, None] - pxo[None, :]
        b6[j] = ((np.abs(rd) <= 1) & (np.abs(wd) <= 1)).astype(np.float16)
    return b6


def kernel(x, images, mu, sigma, t):
    x = np.ascontiguousarray(np.asarray(x, dtype=np.float32))
    images = np.ascontiguousarray(np.asarray(images, dtype=np.float32))
    m = float(np.asarray(mu)[int(t)])
    sig = float(np.asarray(sigma)[int(t)])
    sig2 = sig * sig

    if "nc" not in _cache:
        _cache["nc"] = _build()
    nc = _cache["nc"]

    imgs2 = images.reshape(N, C, P)
    spp = (-(m * m) / (2.0 * sig2)) * np.einsum("ncp,ncp->np", imgs2, imgs2)
    xx = (x.reshape(B, C, H, W) * (m / sig2)).astype(np.float32)
    b6 = _band_stack()

    in_maps = []
    for k in range(NCORES):
        bg, nh = k // GN, k % GN
        nsl = slice(nh * NLOC, (nh + 1) * NLOC)
        # rows[r]: [(c,px), n] ; c<3 = I_c row r, c=3 = S'' row r
        rows = np.empty((H, 4, 32, F), np.float16)
        for c in range(C):
            rows[:, c] = (
                imgs2[nsl, c, :].T.reshape(H, 32, F)
            )
        rows[:, 3] = spp[nsl].T.reshape(H, 32, F)
        arr = np.empty((PT, 128, 3, F), np.float16)
        for c in range(C):
            arr[:, :, c, :] = imgs2[nsl, c, :].T.reshape(PT, 128, F)
        # xs[(c,px), r*NB+b] = xhat_{b,c}[row r, px]; 1.0 for c=3
        xs = np.ones((4, 32, H, NB), np.float32)
        for c in range(C):
            for b in range(NB):
                xs[c, :, :, b] = xx[bg * NB + b, c].T  # [W=32 px, H=32 rows]
        xs = xs.reshape(128, H * NB)
        in_maps.append(
            {
                "rows": np.ascontiguousarray(rows.reshape(H, 128, F)),
                "imgs": np.ascontiguousarray(arr),
                "xs": np.ascontiguousarray(xs),
                "bstk": b6,
            }
        )

    import os

    trace = bool(os.environ.get("KERNEL_TRACE"))
    res = bass_utils.run_bass_kernel_spmd(
        nc, in_maps, core_ids=list(range(NCORES)), trace=trace
    )
    global _last_res
    _last_res = res

    sw = np.zeros((B, P), np.float64)
    swi = np.zeros((B, C, P), np.float64)
    for k in range(NCORES):
        bg = k // GN
        st = np.asarray(res.results[k]["out"], np.float64).reshape(128, NB, PT, 4)
        for b in range(NB):
            gb = bg * NB + b
            sw[gb] += st[:, b, :, 0].T.reshape(P)
            for c in range(C):
                swi[gb, c] += st[:, b, :, 1 + c].T.reshape(P)

    score = (m * swi / sw[:, None, :] - x.reshape(B, C, P)) / sig2
    return score.reshape(B, C, H, W).astype(np.float32)


# revision 5
# speedup vs baseline: 1.1359x; 1.0009x over previous
"""LocalScoreMachine Trainium2 kernel, v3: K-stacked box matmuls.

arg[b, po, n] = sum_{c,pi} B[pi,po] * xhat_bc[pi] * I_c[pi,n]  (+ S'' as c=3,
xhat=1). Rhs shipped as row-ktiles: one SBUF tile per image row r holding
[(4 channels x 32 pixels) = 128 partitions, n]. Each output tile t contracts
its 6 halo rows with 6 per-(b,t) lhsT matrices built by a single 4x
tensor_scalar row-scaling of a constant band pattern. This removes all
z-products and the S'' add from DVE and cuts PE to 6 matmuls per chunk.
w = exp(arg) on ACT (free SW accum); SWI_c via DVE products + ts accums.
Sharding: 4 query-pair groups x 2 dataset halves.
"""

import sys

for _p in ("/opt/trn_rl_repo", "/opt/trn_rl_repo/concourse", "/opt/pypackages"):
    if _p not in sys.path:
        sys.path.append(_p)

from contextlib import ExitStack

import numpy as np

import concourse.bass as bass
import concourse.bacc as bacc
import concourse.mybir as mybir
import concourse.tile as tile
from concourse import bass_utils

B, N, C, H, W = 8, 2048, 3, 32, 32
P = H * W
NCORES = 8
GB, GN = 4, 2
NB = B // GB  # 2
NLOC = N // GN  # 1024
PT = P // 128  # 8
F = NLOC
CH = 512

F32 = mybir.dt.float32
F16 = mybir.dt.float16
AF = mybir.ActivationFunctionType
OP = mybir.AluOpType

ACT_MOD = 2  # every ACT_MOD-th channel reduction goes to DVE, rest to ACT

_cache = {}
_last_res = None


def _build():
    nc = bacc.Bacc("TRN2", target_bir_lowering=False, debug=False)

    rows_d = nc.dram_tensor("rows", [H, 128, F], F16, kind="ExternalInput")
    img_d = nc.dram_tensor("imgs", [PT, 128, 3, F], F16, kind="ExternalInput")
    xs_d = nc.dram_tensor("xs", [128, H * NB], F32, kind="ExternalInput")
    bs_d = nc.dram_tensor("bstk", [6, 128, 128], F16, kind="ExternalInput")
    out_d = nc.dram_tensor("out", [128, NB * PT * 4], F32, kind="ExternalOutput")

    with tile.TileContext(nc) as tc, ExitStack() as ctx:
        const = ctx.enter_context(tc.tile_pool(name="const", bufs=1))
        rpool = ctx.enter_context(tc.tile_pool(name="rpool", bufs=1))
        imgs = ctx.enter_context(tc.tile_pool(name="imgs", bufs=1))
        lp = ctx.enter_context(tc.tile_pool(name="lp", bufs=2))
        wp = ctx.enter_context(tc.tile_pool(name="wp", bufs=3))
        scr = ctx.enter_context(tc.tile_pool(name="scr", bufs=3))
        psum = ctx.enter_context(
            tc.tile_pool(name="psum", bufs=2, space=bass.MemorySpace.PSUM)
        )

        bstk = const.tile([128, 6 * 128], F16)
        nc.scalar.dma_start(
            bstk[:].rearrange("p (k m) -> p k m", k=6),
            bs_d.ap().rearrange("k p m -> p k m"),
        )
        xst = const.tile([128, H * NB], F32)
        nc.scalar.dma_start(xst[:], xs_d.ap())
        stage = const.tile([128, NB * PT * 4], F32)

        # row-ktiles batched 4 rows per DMA (Pool queue); first group split in
        # halves so the tiny bstk/xs transfers are not stuck behind it
        rtiles = []
        itiles = []
        for g in range(PT):
            gt = rpool.tile([128, 4 * F], F16, tag=f"rg{g}", name=f"rg{g}")
            if g < 3:
                for h in range(2):
                    nc.gpsimd.dma_start(
                        gt[:, h * 2 * F : (h + 1) * 2 * F].rearrange(
                            "p (r n) -> p r n", r=2
                        ),
                        rows_d.ap()[4 * g + 2 * h : 4 * g + 2 * h + 2].rearrange(
                            "r p n -> p r n"
                        ),
                    )
            else:
                nc.gpsimd.dma_start(
                    gt[:].rearrange("p (r n) -> p r n", r=4),
                    rows_d.ap()[4 * g : 4 * g + 4].rearrange("r p n -> p r n"),
                )
            for j in range(4):
                rtiles.append(gt[:, j * F : (j + 1) * F])
            it = imgs.tile([128, 3 * F], F16, tag=f"img{g}", name=f"img{g}")
            itiles.append(it)

        def emit_img_dma(g):
            nc.scalar.dma_start(
                itiles[g][:].rearrange("p (c n) -> p c n", c=3), img_d.ap()[g]
            )

        def xcol(b, r):
            j = r * NB + b
            return xst[:, j : j + 1]

        def rows_of(t):
            return [r for r in range(4 * t - 1, 4 * t + 5) if 0 <= r < H]

        lts = [[None] * PT for _ in range(NB)]

        def emit_lhs(b, t):
            ls = []
            for r in rows_of(t):
                j = r - (4 * t - 1)  # band-pattern index 0..5
                L = lp.tile([128, 128], F16, tag=f"L{b}_{j}", name=f"L{b}_{j}_{t}")
                nc.vector.tensor_scalar_mul(
                    L[:], bstk[:, j * 128 : (j + 1) * 128], xcol(b, r)
                )
                ls.append((r, L))
            lts[b][t] = ls

        def scol(b, t, j):
            k = b * (PT * 4) + t * 4 + j
            return stage[:, k : k + 1]

        wts = [[None] * PT for _ in range(NB)]

        def emit_boxexp(b, t):
            ls = lts[b][t]
            ps = psum.tile([128, F], F32, tag=f"ps{b}", name=f"ps_{b}_{t}")
            n_mm = len(ls) * 2
            i = 0
            for r, L in ls:
                for ck in range(2):
                    sl = slice(ck * CH, (ck + 1) * CH)
                    nc.tensor.matmul(
                        ps[:, sl],
                        L[:],
                        rtiles[r][:, sl.start : sl.stop] if False else rtiles[r][:, sl],
                        start=(i < 2),
                        stop=(i >= n_mm - 2),
                        skip_group_check=True,
                    )
                    i += 1
            w = wp.tile([128, F], F16, tag=f"w{b}", name=f"w_{b}_{t}")
            nc.scalar.activation(w[:], ps[:], AF.Exp, accum_out=scol(b, t, 0))
            wts[b][t] = w

        rctr = [0]

        def emit_red(b, t, kact=None):
            it = itiles[t]
            w = wts[b][t]
            for c in range(C):
                v = scr.tile([128, F], F16, tag=f"v_{b}", name=f"v{c}_{b}_{t}")
                nc.vector.tensor_tensor(
                    v[:], w[:], it[:, c * F : (c + 1) * F], OP.mult
                )
                rctr[0] += 1
                use_act = (rctr[0] % ACT_MOD) != 0 if kact is None else False
                if use_act:
                    d = scr.tile([128, F], F16, tag="da", name=f"d{c}_{b}_{t}")
                    nc.scalar.activation(
                        d[:], v[:], AF.Copy, accum_out=scol(b, t, 1 + c)
                    )
                else:
                    d = scr.tile([128, F], F16, tag="dv", name=f"d{c}_{b}_{t}")
                    nc.vector.tensor_scalar(
                        d[:], v[:], 1.0, 0.0, OP.mult, OP.add,
                        accum_out=scol(b, t, 1 + c),
                    )

        steps = [(t, b) for t in range(PT) for b in range(NB)]
        nst = len(steps)
        for k in range(nst + 4):
            if k < nst:
                t, b = steps[k]
                if b == 0:
                    emit_img_dma(t)
                emit_lhs(b, t)
            if 2 <= k < nst + 2:
                t1_, b1_ = steps[k - 2]
                emit_boxexp(b1_, t1_)
            if 4 <= k < nst + 4:
                t2_, b2_ = steps[k - 4]
                emit_red(b2_, t2_, kact=0 if k >= nst + 2 else None)

        nc.sync.dma_start(out_d.ap(), stage[:])

    nc.compile()
    return nc


def _band_stack():
    # bstk[j][(c,px_in), po] : band for input row-offset j-1 vs output row po//32
    px_i = np.arange(128) % 32  # within each 32-px channel group
    po = np.arange(128)
    ro, pxo = po // 32, po % 32
    b6 = np.zeros((6, 128, 128), np.float16)
    for j in range(6):
        rd = (j - 1) - ro[None, :]
        wd = px_i[:, None] - pxo[None, :]
        b6[j] = ((np.abs(rd) <= 1) & (np.abs(wd) <= 1)).astype(np.float16)
    return b6


def kernel(x, images, mu, sigma, t):
    x = np.ascontiguousarray(np.asarray(x, dtype=np.float32))
    images = np.ascontiguousarray(np.asarray(images, dtype=np.float32))
    m = float(np.asarray(mu)[int(t)])
    sig = float(np.asarray(sigma)[int(t)])
    sig2 = sig * sig

    if "nc" not in _cache:
        _cache["nc"] = _build()
    nc = _cache["nc"]

    imgs2 = images.reshape(N, C, P)
    spp = (-(m * m) / (2.0 * sig2)) * np.einsum("ncp,ncp->np", imgs2, imgs2)
    xx = (x.reshape(B, C, H, W) * (m / sig2)).astype(np.float32)
    b6 = _band_stack()

    in_maps = []
    for k in range(NCORES):
        bg, nh = k // GN, k % GN
        nsl = slice(nh * NLOC, (nh + 1) * NLOC)
        # rows[r]: [(c,px), n] ; c<3 = I_c row r, c=3 = S'' row r
        rows = np.empty((H, 4, 32, F), np.float16)
        for c in range(C):
            rows[:, c] = (
                imgs2[nsl, c, :].T.reshape(H, 32, F)
            )
        rows[:, 3] = spp[nsl].T.reshape(H, 32, F)
        arr = np.empty((PT, 128, 3, F), np.float16)
        for c in range(C):
            arr[:, :, c, :] = imgs2[nsl, c, :].T.reshape(PT, 128, F)
        # xs[(c,px), r*NB+b] = xhat_{b,c}[row r, px]; 1.0 for c=3
        xs = np.ones((4, 32, H, NB), np.float32)
        for c in range(C):
            for b in range(NB):
                xs[c, :, :, b] = xx[bg * NB + b, c].T  # [W=32 px, H=32 rows]
        xs = xs.reshape(128, H * NB)
        in_maps.append(
            {
                "rows": np.ascontiguousarray(rows.reshape(H, 128, F)),
                "imgs": np.ascontiguousarray(arr),
                "xs": np.ascontiguousarray(xs),
                "bstk": b6,
            }
        )

    import os

    trace = bool(os.environ.get("KERNEL_TRACE"))
    res = bass_utils.run_bass_kernel_spmd(
        nc, in_maps, core_ids=list(range(NCORES)), trace=trace
    )
    global _last_res
    _last_res = res

    sw = np.zeros((B, P), np.float64)
    swi = np.zeros((B, C, P), np.float64)
    for k in range(NCORES):
        bg = k // GN
        st = np.asarray(res.results[k]["out"], np.float64).reshape(128, NB, PT, 4)
        for b in range(NB):
            gb = bg * NB + b
            sw[gb] += st[:, b, :, 0].T.reshape(P)
            for c in range(C):
                swi[gb, c] += st[:, b, :, 1 + c].T.reshape(P)

    score = (m * swi / sw[:, None, :] - x.reshape(B, C, P)) / sig2
    return score.reshape(B, C, H, W).astype(np.float32)
